# revision 19
# baseline (speedup 1.0000x reference)
"""DCGRU cell on 8 Trainium2 NeuronCores (Bass/Tile), v4.

Math: with a = adj + I, d = a.sum(axis=1), T = (d^-1 a)^T, every
diffusion step is  y = T @ v = a^T @ (d_inv * v).  d_inv is computed on
the HOST, so there are no row-sum collectives.  The d_inv factor rides
on the activation side: the stationary operand of each diffusion matmul
is z = c * d_inv * v (c a power of 2 keeping fp8 values in normal
range).  All unscale constants fold into the host-prepared gate weights
(W0' = W0 - W2, W1' = W1/c0, W2' = 2*W2/c1), so diffusion PSUMs are
evacuated as raw bf16 copies and the Chebyshev combine x2 = 2*T@x1 - x0
never materializes.

Sharding (8 cores): 1D column-parallel over the adjacency.  Core m
holds a[:, m*1024:(m+1)*1024] as fp8e4 (8 MB), host-permuted p-major so
partition lines are contiguous.  Each diffusion is a DoubleRow fp8
matmul: stationary z pair-chunk [128, 2, 72], moving adjacency
[128, 2, 512] -> psum [72, 512].

v4 vs v3: the whole z / diffusion-output / AllGather path is PACKED to
72 features (b*18+j) instead of padded to 128 (b*32+j) — AG payloads
shrink 44% (the AG is the dominant inter-diffusion cost).  DoubleRow's
Ko-step%16 rule rules out a 72-stride pair (k, k+1), so chunk pairs are
remapped to (k, k+2) (step 144 = 9*16) — both chunks still live in the
same streamed adjacency tile.  Gate matmuls bridge the packed (m1/m2
sources) and padded (m0 source, psum output b*32+u) layouts with
host-built block-diagonal [72, 128] stationaries — one matmul per term
instead of four.  A tiny warm-up AllGather fires at kernel start so the
first real AG doesn't pay the ~11 us ncfw arming delay.
"""

import numpy as np
import ml_dtypes

import concourse.bass as bass
import concourse.bacc as bacc
import concourse.tile as tile
import concourse.mybir as mybir
import concourse.bass_utils as bass_utils

F32 = mybir.dt.float32
BF16 = mybir.dt.bfloat16
FP8 = mybir.dt.float8e4
AF = mybir.ActivationFunctionType
ALU = mybir.AluOpType
DR = mybir.MatmulPerfMode.DoubleRow

NCORES = 8
N = 8192          # nodes
C = N // NCORES   # own nodes per core (1024)
P = 128           # partitions
KCH = N // P      # node chunks (64)
NT = 16           # streamed adjacency tiles
CPI = KCH // NT   # chunks per stream tile (4)
KP = KCH // 2     # DoubleRow chunk pairs (32)
MB = C // P       # own-node tiles (8)
B = 4             # batch
IT = 18           # I_tot = in_dim + units
FT = B * IT       # packed feature width (72)
FPAD = P          # z-tile feature stride (128; cols 72..127 unused)
U = 16            # units
IN_DIM = 2
HC = C // 2       # half own-node width (512)

C0 = 4096.0       # z0 = C0 * d_inv * x0     (fp8-range normalizer)
C1 = 262144.0     # z1 = C1 * d_inv * x1
S0 = 1.0 / C0
S1 = 2.0 / C1

_CACHE = {}


def _build():
    nc = bacc.Bacc("TRN2", target_bir_lowering=False, debug=False,
                   num_devices=NCORES)

    a_d = nc.dram_tensor("a", [P, KCH, C], FP8, kind="ExternalInput")
    z0_d = nc.dram_tensor("z0", [P, KCH, FT], FP8, kind="ExternalInput")
    x0T_d = nc.dram_tensor("x0T", [P, C], BF16, kind="ExternalInput")
    hxT_d = nc.dram_tensor("hxT", [P, C], BF16, kind="ExternalInput")
    wr0_d = nc.dram_tensor("wr0", [P, 32], BF16, kind="ExternalInput")
    wu0_d = nc.dram_tensor("wu0", [P, 32], BF16, kind="ExternalInput")
    wc0_d = nc.dram_tensor("wc0", [P, 32], BF16, kind="ExternalInput")
    wr12_d = nc.dram_tensor("wr12", [FT, 2, P], BF16, kind="ExternalInput")
    wu12_d = nc.dram_tensor("wu12", [FT, 2, P], BF16, kind="ExternalInput")
    wc12_d = nc.dram_tensor("wc12", [FT, 2, P], BF16, kind="ExternalInput")
    brur_d = nc.dram_tensor("brur", [P, 1], F32, kind="ExternalInput")
    bruu_d = nc.dram_tensor("bruu", [P, 1], F32, kind="ExternalInput")
    bc_d = nc.dram_tensor("bc", [P, 1], F32, kind="ExternalInput")
    cdzA_d = nc.dram_tensor("cdzA", [P, MB], BF16, kind="ExternalInput")
    cdzB_d = nc.dram_tensor("cdzB", [P, MB], BF16, kind="ExternalInput")
    ident_d = nc.dram_tensor("ident", [P, P], BF16, kind="ExternalInput")
    out_d = nc.dram_tensor("out", [P, C], BF16, kind="ExternalOutput")

    with tile.TileContext(nc) as tc:
        with (
            tc.tile_pool(name="big", bufs=1) as big,
            tc.tile_pool(name="psmm", bufs=2, space="PSUM") as psmm,
            tc.tile_pool(name="pstp", bufs=2, space="PSUM") as pstp,
            tc.tile_pool(name="psg", bufs=4, space="PSUM") as psg,
            tc.tile_pool(name="dram", bufs=1, space="DRAM") as dram,
        ):
            # ---------- persistent SBUF tensors ----------
            abf = [big.tile([P, CPI, C], FP8, name=f"abf{i}")
                   for i in range(NT)]

            z0 = big.tile([P, KCH, FT], FP8)
            zgA = big.tile([P, KCH, FT], FP8)    # gathered z1; later z1c
            zgB = big.tile([P, KCH, FT], FP8)    # gathered z0c
            x0T = big.tile([P, C], BF16)         # padded; becomes x0cT
            hxT = big.tile([P, C], BF16)
            y1raw = big.tile([P, C], BF16)       # rows 0..71 packed features
            x2raw = big.tile([P, C], BF16)
            y1craw = big.tile([P, C], BF16)
            x2craw = big.tile([P, C], BF16)
            sigR = big.tile([P, C], BF16)
            sigU = big.tile([P, C], BF16)
            cT = big.tile([P, C], BF16)
            outT = big.tile([P, C], BF16)
            wr0 = big.tile([P, 32], BF16)
            wu0 = big.tile([P, 32], BF16)
            wc0 = big.tile([P, 32], BF16)
            wr12 = big.tile([FT, 2, P], BF16)
            wu12 = big.tile([FT, 2, P], BF16)
            wc12 = big.tile([FT, 2, P], BF16)
            brur = big.tile([P, 1], F32)
            bruu = big.tile([P, 1], F32)
            bc = big.tile([P, 1], F32)
            cdzA = big.tile([P, MB], BF16)
            cdzB = big.tile([P, MB], BF16)
            identbf = big.tile([P, P], BF16)
            pkA = big.tile([P, MB, FT], FP8)     # packed AG payloads
            pkB = big.tile([P, MB, B, IT], FP8)
            pkC = big.tile([P, MB, FT], FP8)

            wsrc = big.tile([P, 16], FP8)

            # warm-up collective: identical shape to the real AllGathers,
            # fired during the stream phase so the first real AG doesn't
            # pay the ~15 us ncfw arming delay
            nc.gpsimd.memset(wsrc[:], 0)
            aginW = dram.tile([P, 16], FP8, tag="aginW")
            agoutW = dram.tile([NCORES, P, 16], FP8,
                               addr_space="Shared", tag="agoutW")
            nc.gpsimd.dma_start(aginW[:], wsrc[:])
            nc.gpsimd.collective_compute(
                "AllGather", ALU.bypass,
                replica_groups=[list(range(NCORES))],
                ins=[aginW[:]], outs=[agoutW[:]],
            )

            # pad rows of the raw-output tiles are read (as garbage) by
            # the full-128 transposes; define them once (base must be
            # 32-aligned, so start at 64 — rows 64..71 are re-written by
            # every psum evacuation afterwards)
            for t in (y1raw, x2raw, y1craw, x2craw):
                nc.vector.memset(t[64:P, :], 0)


            # ---------- input DMAs ----------
            nc.scalar.dma_start(z0[:], z0_d[:])
            nc.scalar.dma_start(x0T[:], x0T_d[:])
            nc.scalar.dma_start(hxT[:], hxT_d[:])
            for t, s in ((wr0, wr0_d), (wu0, wu0_d), (wc0, wc0_d),
                         (wr12, wr12_d), (wu12, wu12_d), (wc12, wc12_d),
                         (brur, brur_d), (bruu, bruu_d), (bc, bc_d),
                         (cdzA, cdzA_d), (cdzB, cdzB_d),
                         (identbf, ident_d)):
                nc.gpsimd.dma_start(t[:], s[:])

            # ---------- adjacency stream (fp8, p-major contiguous) ----
            S, G, Csc = nc.sync, nc.gpsimd, nc.scalar
            ENGS = [S, G, Csc, S, G, S, G, Csc, S, G, S, Csc, G, S, G, Csc]
            for i in range(NT):
                ENGS[i].dma_start(abf[i][:], a_d[:, i * CPI:(i + 1) * CPI, :])

            # DoubleRow chunk pairs: (4i+lo, 4i+2+lo) — both chunks in
            # stream tile i, Ko step 2*FT = 144 bytes (16-aligned), so the
            # z tiles stay tight 72-wide and the gather lands contiguously.
            def zpair(zt, i, lo):
                v = zt[:].rearrange("p (i hi lo) f -> p i hi lo f",
                                    hi=2, lo=2)
                return v[:, i, :, lo, :]

            def apair(i, lo, h):
                v = abf[i][:].rearrange("p (hi lo) c -> p hi lo c", hi=2)
                return v[:, :, lo, h * HC:(h + 1) * HC]

            def mm_half(ps, zt, h):
                for i in range(NT):
                    for lo in range(2):
                        nc.tensor.matmul(
                            ps[:], lhsT=zpair(zt, i, lo),
                            rhs=apair(i, lo, h),
                            start=(i == 0 and lo == 0),
                            stop=(i == NT - 1 and lo == 1),
                            perf_mode=DR,
                        )

            def transposes_packed(srcT, pk, cdz, mbs):
                """[P, C] node-block (rows 0..71 packed) -> [P, 72], *cdz."""
                for mb in mbs:
                    pt = pstp.tile([P, P], BF16, tag="tp")
                    nc.tensor.transpose(
                        pt[:], srcT[:, mb * P:(mb + 1) * P], identbf[:])
                    cdb = cdz[:, mb:mb + 1].broadcast_to((P, FT))
                    nc.vector.tensor_tensor(
                        pk[:, mb, :], pt[:, 0:FT], cdb, ALU.mult)

            def transposes_padded(srcT, pk, cdz, mbs):
                """Padded [P, C] node-block -> packed payload, *cdz."""
                for mb in mbs:
                    pt = pstp.tile([P, P], BF16, tag="tp")
                    nc.tensor.transpose(
                        pt[:], srcT[:, mb * P:(mb + 1) * P], identbf[:])
                    cdb = cdz[:, mb:mb + 1].unsqueeze(-1).broadcast_to(
                        (P, B, IT))
                    src = pt[:].rearrange("p (b e) -> p b e", b=B)[:, :, 0:IT]
                    nc.vector.tensor_tensor(
                        pk[:, mb, :, :], src, cdb, ALU.mult)

            def allgather(pk, zdst, name):
                agin = dram.tile([P, MB * FT], FP8, tag=f"agin{name}")
                agout = dram.tile([NCORES, P, MB * FT], FP8,
                                  addr_space="Shared", tag=f"agout{name}")
                nc.gpsimd.dma_start(
                    agin[:], pk[:].rearrange("p ... -> p (...)"))
                nc.gpsimd.collective_compute(
                    "AllGather", ALU.bypass,
                    replica_groups=[list(range(NCORES))],
                    ins=[agin[:]], outs=[agout[:]],
                )
                gengs = (nc.sync, nc.sync, nc.sync, nc.scalar, nc.scalar,
                         nc.scalar, nc.gpsimd, nc.gpsimd)
                for r in range(NCORES):
                    gengs[r].dma_start(
                        zdst[:, r * MB:(r + 1) * MB, :],
                        agout[r].rearrange("p (m f) -> p m f", m=MB),
                    )

            def gate_m0(w0, pg, h):
                fs = slice(h * HC, (h + 1) * HC)
                for b in range(B):
                    nc.tensor.matmul(
                        pg[b * 32:(b + 1) * 32, :],
                        lhsT=w0[b * 32:b * 32 + IT, :],
                        rhs=x0T[b * 32:b * 32 + IT, fs],
                        start=True, stop=False,
                        tile_position=(b * 32, b * 32),
                    )

            def gate_m12(w12, m, pg, src, h, stop):
                fs = slice(h * HC, (h + 1) * HC)
                nc.tensor.matmul(
                    pg[:], lhsT=w12[:, m, :], rhs=src[0:FT, fs],
                    start=False, stop=stop,
                )

            # ================= gconv 1 (r/u gates) =================
            # diff A chases the stream (h-inner: consume tiles as they land)
            psA = [psmm.tile([FT, HC], F32, tag="mm", name=f"psA{h}")
                   for h in range(2)]
            for i in range(NT):
                for lo in range(2):
                    for h in range(2):
                        nc.tensor.matmul(
                            psA[h][:], lhsT=zpair(z0, i, lo),
                            rhs=apair(i, lo, h),
                            start=(i == 0 and lo == 0),
                            stop=(i == NT - 1 and lo == 1),
                            perf_mode=DR,
                        )
            for h in range(2):
                nc.vector.tensor_copy(
                    y1raw[0:FT, h * HC:(h + 1) * HC], psA[h][:])
            transposes_packed(y1raw, pkA, cdzA, range(MB))
            allgather(pkA, zgA, "A")
            # r/u gate m0+m1 run inside the AG window
            pg_r = [psg.tile([P, HC], F32, tag="gate", name=f"pgr{h}",
                             bufs=4) for h in range(2)]
            pg_u = [psg.tile([P, HC], F32, tag="gate", name=f"pgu{h}",
                             bufs=4) for h in range(2)]
            for h in range(2):
                gate_m0(wr0, pg_r[h], h)
                gate_m12(wr12, 0, pg_r[h], y1raw, h, False)
                gate_m0(wu0, pg_u[h], h)
                gate_m12(wu12, 0, pg_u[h], y1raw, h, False)

            # diff B (h-outer: half-0 tail overlaps half-1 matmuls)
            psB = [psmm.tile([FT, HC], F32, tag="mm", name=f"psB{h}")
                   for h in range(2)]
            for h in range(2):
                fs = slice(h * HC, (h + 1) * HC)
                mm_half(psB[h], zgA, h)
                nc.vector.tensor_copy(x2raw[0:FT, fs], psB[h][:])
                gate_m12(wr12, 1, pg_r[h], x2raw, h, True)
                gate_m12(wu12, 1, pg_u[h], x2raw, h, True)
                nc.scalar.activation(sigR[:, fs], pg_r[h][:], AF.Sigmoid,
                                     bias=brur[:])
                for b in range(B):
                    nc.vector.tensor_tensor(
                        x0T[b * 32:b * 32 + U, fs],
                        sigR[b * 32:b * 32 + U, fs],
                        hxT[b * 32:b * 32 + U, fs],
                        ALU.mult,
                    )
                transposes_padded(x0T, pkB, cdzB, range(h * MB // 2,
                                                        (h + 1) * MB // 2))
                nc.scalar.activation(sigU[:, fs], pg_u[h][:], AF.Sigmoid,
                                     bias=bruu[:])
            allgather(pkB, zgB, "B")
            # c gate m0 fills part of the AG_B window (x0T is x0cT now)
            pg_c = [psg.tile([P, HC], F32, tag="gate", name=f"pgc{h}",
                             bufs=4) for h in range(2)]
            for h in range(2):
                gate_m0(wc0, pg_c[h], h)

            # ================= gconv 2 (candidate c) =================
            psC = [psmm.tile([FT, HC], F32, tag="mm", name=f"psC{h}")
                   for h in range(2)]
            for h in range(2):
                fs = slice(h * HC, (h + 1) * HC)
                mm_half(psC[h], zgB, h)
                nc.vector.tensor_copy(y1craw[0:FT, fs], psC[h][:])
                transposes_packed(y1craw, pkC, cdzA,
                                  range(h * MB // 2, (h + 1) * MB // 2))
            allgather(pkC, zgA, "C")
            for h in range(2):
                gate_m12(wc12, 0, pg_c[h], y1craw, h, False)

            # diff D + per-half tail to the output DMA
            psD = [psmm.tile([FT, HC], F32, tag="mm", name=f"psD{h}")
                   for h in range(2)]
            for h in range(2):
                fs = slice(h * HC, (h + 1) * HC)
                mm_half(psD[h], zgA, h)
                nc.vector.tensor_copy(x2craw[0:FT, fs], psD[h][:])
                gate_m12(wc12, 1, pg_c[h], x2craw, h, True)
                nc.scalar.activation(cT[:, fs], pg_c[h][:], AF.Tanh,
                                     bias=bc[:])
                # out = c + u*(h - c)
                eng = nc.gpsimd if h == 0 else nc.vector
                eng.tensor_tensor(outT[:, fs], hxT[:, fs], cT[:, fs],
                                  ALU.subtract)
                eng.tensor_tensor(outT[:, fs], outT[:, fs], sigU[:, fs],
                                  ALU.mult)
                eng.tensor_tensor(outT[:, fs], outT[:, fs], cT[:, fs],
                                  ALU.add)
                (nc.sync if h == 0 else nc.scalar).dma_start(
                    out_d[:, fs], outT[:, fs])

    nc.compile()
    return nc


def _get_nc():
    if "nc" not in _CACHE:
        _CACHE["nc"] = _build()
    return _CACHE["nc"]


# feature permutation: device feature j -> reference feature i
# j = 0..15 -> i = j+2 (hidden), j = 16,17 -> i = j-16 (input x)
_PERM = np.array(list(range(2, 18)) + [0, 1])


def _host_prep(inputs, hx, adj, W_ru, b_ru, W_c, b_c):
    f32 = np.float32
    bf16 = ml_dtypes.bfloat16
    fp8 = ml_dtypes.float8_e4m3fn

    xr = np.ascontiguousarray(inputs, dtype=f32).reshape(B, N, IN_DIM)
    hr = np.ascontiguousarray(hx, dtype=f32).reshape(B, N, U)
    x0 = np.concatenate([hr, xr], axis=2).transpose(1, 0, 2)  # [N, B, 18]
    x0 = np.ascontiguousarray(x0).reshape(N, FT)

    adj = np.asarray(adj, f32)
    d = adj.sum(axis=1) + 1.0
    dinv = 1.0 / d

    # z0 = C0 * dinv * x0, tight 72-wide, p-major, fp8
    z0 = (C0 * dinv)[:, None] * x0                  # [N, 72]
    z0 = z0.reshape(KCH, P, FT).transpose(1, 0, 2)  # [P, KCH, FT]
    z0 = np.ascontiguousarray(z0).astype(fp8)

    def w_split(w, lo):
        # fold the diffusion unscales into the weights:
        # gate = x0*(W0-W2) + (c0*x1)*(W1/c0) + (c1/2*Tx1)*(2*W2/c1)
        w3 = np.asarray(w, f32).reshape(IT, 3, -1)[_PERM][:, :, lo:lo + U]
        w0 = w3[:, 0] - w3[:, 2]                    # [18, U]
        m0 = np.zeros((B, 32, 32), f32)
        m0[:, 0:IT, 0:U] = w0[None]
        w1 = S0 * w3[:, 1]
        w2 = S1 * w3[:, 2]
        m12 = np.zeros((B, IT, 2, B, 32), f32)
        for b in range(B):
            m12[b, :, 0, b, 0:U] = w1
            m12[b, :, 1, b, 0:U] = w2
        return (m0.reshape(P, 32).astype(bf16),
                m12.reshape(FT, 2, P).astype(bf16))

    wr0_p, wr12_p = w_split(W_ru, 0)
    wu0_p, wu12_p = w_split(W_ru, U)
    wc0_p, wc12_p = w_split(W_c, 0)

    def pad_bias(v):
        t = np.zeros((B, 32), f32)
        t[:, 0:U] = np.asarray(v, f32)
        return np.ascontiguousarray(t.reshape(P)[:, None])

    brur_t = pad_bias(np.asarray(b_ru, f32)[0:U])
    bruu_t = pad_bias(np.asarray(b_ru, f32)[U:2 * U])
    bc_t = pad_bias(np.asarray(b_c, f32))
    ident = np.eye(P, dtype=f32).astype(bf16)

    in_maps = []
    for m in range(NCORES):
        sl = slice(m * C, (m + 1) * C)
        a_m = np.ascontiguousarray(adj[:, sl])
        a_m[m * C + np.arange(C), np.arange(C)] += 1.0
        # p-major: [p, k, c] = row k*128+p
        a_m = a_m.reshape(KCH, P, C).transpose(1, 0, 2)
        a_m = np.ascontiguousarray(a_m).astype(fp8)

        x0own = x0[sl]
        x0T = np.zeros((B, 32, C), f32)
        x0T[:, 0:IT, :] = x0own.reshape(C, B, IT).transpose(1, 2, 0)
        x0T = x0T.reshape(P, C).astype(bf16)
        hxT_p = np.zeros((B, 32, C), f32)
        hxT_p[:, 0:U, :] = hr[:, sl, :].transpose(0, 2, 1)
        hxT_p = hxT_p.reshape(P, C).astype(bf16)
        # per-own-node payload scales, node-major [p, mb]
        dlocal = dinv[sl].reshape(MB, P).T
        cdzA_p = np.ascontiguousarray((C1 / C0) * dlocal).astype(bf16)
        cdzB_p = np.ascontiguousarray(C0 * dlocal).astype(bf16)
        in_maps.append({
            "a": a_m,
            "z0": z0,
            "x0T": x0T,
            "hxT": hxT_p,
            "wr0": wr0_p,
            "wu0": wu0_p,
            "wc0": wc0_p,
            "wr12": wr12_p,
            "wu12": wu12_p,
            "wc12": wc12_p,
            "brur": brur_t,
            "bruu": bruu_t,
            "bc": bc_t,
            "cdzA": cdzA_p,
            "cdzB": cdzB_p,
            "ident": ident,
        })
    return in_maps


def _run(in_maps, trace=False, **kw):
    nc = _get_nc()
    return bass_utils.run_bass_kernel_spmd(
        nc, in_maps, core_ids=list(range(NCORES)), trace=trace, **kw)


def _assemble(results):
    out = np.empty((B, N * U), np.float32)
    for m in range(NCORES):
        # device layout [b*32+u, n] (rows 16..31 per block are padding)
        blk = results[m]["out"].astype(np.float32).reshape(B, 32, C)[:, 0:U, :].transpose(0, 2, 1)
        out[:, m * C * U:(m + 1) * C * U] = blk.reshape(B, C * U)
    return out


def kernel(inputs, hx, adj, W_ru, b_ru, W_c, b_c):
    in_maps = _host_prep(inputs, hx, adj, W_ru, b_ru, W_c, b_c)
    res = _run(in_maps)
    return _assemble(res.results)


# revision 20
# speedup vs baseline: 1.0168x; 1.0168x over previous
"""DCGRU cell on 8 Trainium2 NeuronCores (Bass/Tile), v4.

Math: with a = adj + I, d = a.sum(axis=1), T = (d^-1 a)^T, every
diffusion step is  y = T @ v = a^T @ (d_inv * v).  d_inv is computed on
the HOST, so there are no row-sum collectives.  The d_inv factor rides
on the activation side: the stationary operand of each diffusion matmul
is z = c * d_inv * v (c a power of 2 keeping fp8 values in normal
range).  All unscale constants fold into the host-prepared gate weights
(W0' = W0 - W2, W1' = W1/c0, W2' = 2*W2/c1), so diffusion PSUMs are
evacuated as raw bf16 copies and the Chebyshev combine x2 = 2*T@x1 - x0
never materializes.

Sharding (8 cores): 1D column-parallel over the adjacency.  Core m
holds a[:, m*1024:(m+1)*1024] as fp8e4 (8 MB), host-permuted p-major so
partition lines are contiguous.  Each diffusion is a DoubleRow fp8
matmul: stationary z pair-chunk [128, 2, 72], moving adjacency
[128, 2, 512] -> psum [72, 512].

v4 vs v3: the whole z / diffusion-output / AllGather path is PACKED to
72 features (b*18+j) instead of padded to 128 (b*32+j) — AG payloads
shrink 44% (the AG is the dominant inter-diffusion cost).  DoubleRow's
Ko-step%16 rule rules out a 72-stride pair (k, k+1), so chunk pairs are
remapped to (k, k+2) (step 144 = 9*16) — both chunks still live in the
same streamed adjacency tile.  Gate matmuls bridge the packed (m1/m2
sources) and padded (m0 source, psum output b*32+u) layouts with
host-built block-diagonal [72, 128] stationaries — one matmul per term
instead of four.  A tiny warm-up AllGather fires at kernel start so the
first real AG doesn't pay the ~11 us ncfw arming delay.
"""

import numpy as np
import ml_dtypes

import concourse.bass as bass
import concourse.bacc as bacc
import concourse.tile as tile
import concourse.mybir as mybir
import concourse.bass_utils as bass_utils

F32 = mybir.dt.float32
BF16 = mybir.dt.bfloat16
FP8 = mybir.dt.float8e4
AF = mybir.ActivationFunctionType
ALU = mybir.AluOpType
DR = mybir.MatmulPerfMode.DoubleRow

NCORES = 8
N = 8192          # nodes
C = N // NCORES   # own nodes per core (1024)
P = 128           # partitions
KCH = N // P      # node chunks (64)
NT = 16           # streamed adjacency tiles
CPI = KCH // NT   # chunks per stream tile (4)
KP = KCH // 2     # DoubleRow chunk pairs (32)
MB = C // P       # own-node tiles (8)
B = 4             # batch
IT = 18           # I_tot = in_dim + units
FT = B * IT       # packed feature width (72)
FPAD = P          # z-tile feature stride (128; cols 72..127 unused)
U = 16            # units
IN_DIM = 2
HC = C // 2       # half own-node width (512)

C0 = 4096.0       # z0 = C0 * d_inv * x0     (fp8-range normalizer)
C1 = 262144.0     # z1 = C1 * d_inv * x1
S0 = 1.0 / C0
S1 = 2.0 / C1

_CACHE = {}


def _build():
    nc = bacc.Bacc("TRN2", target_bir_lowering=False, debug=False,
                   num_devices=NCORES)

    a_d = nc.dram_tensor("a", [P, KCH, C], FP8, kind="ExternalInput")
    z0_d = nc.dram_tensor("z0", [P, KCH, FT], FP8, kind="ExternalInput")
    x0T_d = nc.dram_tensor("x0T", [P, C], BF16, kind="ExternalInput")
    hxT_d = nc.dram_tensor("hxT", [P, C], BF16, kind="ExternalInput")
    wr0_d = nc.dram_tensor("wr0", [P, 32], BF16, kind="ExternalInput")
    wu0_d = nc.dram_tensor("wu0", [P, 32], BF16, kind="ExternalInput")
    wc0_d = nc.dram_tensor("wc0", [P, 32], BF16, kind="ExternalInput")
    wr12_d = nc.dram_tensor("wr12", [FT, 2, P], BF16, kind="ExternalInput")
    wu12_d = nc.dram_tensor("wu12", [FT, 2, P], BF16, kind="ExternalInput")
    wc12_d = nc.dram_tensor("wc12", [FT, 2, P], BF16, kind="ExternalInput")
    brur_d = nc.dram_tensor("brur", [P, 1], F32, kind="ExternalInput")
    bruu_d = nc.dram_tensor("bruu", [P, 1], F32, kind="ExternalInput")
    bc_d = nc.dram_tensor("bc", [P, 1], F32, kind="ExternalInput")
    cdzA_d = nc.dram_tensor("cdzA", [P, MB], BF16, kind="ExternalInput")
    cdzB_d = nc.dram_tensor("cdzB", [P, MB], BF16, kind="ExternalInput")
    ident_d = nc.dram_tensor("ident", [P, P], BF16, kind="ExternalInput")
    out_d = nc.dram_tensor("out", [P, C], BF16, kind="ExternalOutput")

    with tile.TileContext(nc) as tc:
        with (
            tc.tile_pool(name="big", bufs=1) as big,
            tc.tile_pool(name="psmm", bufs=2, space="PSUM") as psmm,
            tc.tile_pool(name="pstp", bufs=2, space="PSUM") as pstp,
            tc.tile_pool(name="psg", bufs=4, space="PSUM") as psg,
            tc.tile_pool(name="dram", bufs=1, space="DRAM") as dram,
        ):
            # ---------- persistent SBUF tensors ----------
            abf = [big.tile([P, CPI, C], FP8, name=f"abf{i}")
                   for i in range(NT)]

            z0 = big.tile([P, KCH, FT], FP8)
            zgA = big.tile([P, KCH, FT], FP8)    # gathered z1; later z1c
            zgB = big.tile([P, KCH, FT], FP8)    # gathered z0c
            x0T = big.tile([P, C], BF16)         # padded; becomes x0cT
            hxT = big.tile([P, C], BF16)
            y1raw = big.tile([P, C], BF16)       # rows 0..71 packed features
            x2raw = big.tile([P, C], BF16)
            y1craw = big.tile([P, C], BF16)
            x2craw = big.tile([P, C], BF16)
            sigR = big.tile([P, C], BF16)
            sigU = big.tile([P, C], BF16)
            cT = big.tile([P, C], BF16)
            outT = big.tile([P, C], BF16)
            wr0 = big.tile([P, 32], BF16)
            wu0 = big.tile([P, 32], BF16)
            wc0 = big.tile([P, 32], BF16)
            wr12 = big.tile([FT, 2, P], BF16)
            wu12 = big.tile([FT, 2, P], BF16)
            wc12 = big.tile([FT, 2, P], BF16)
            brur = big.tile([P, 1], F32)
            bruu = big.tile([P, 1], F32)
            bc = big.tile([P, 1], F32)
            cdzA = big.tile([P, MB], BF16)
            cdzB = big.tile([P, MB], BF16)
            identbf = big.tile([P, P], BF16)
            pkA = big.tile([P, MB, FT], FP8)     # packed AG payloads
            pkB = big.tile([P, MB, B, IT], FP8)
            pkC = big.tile([P, MB, FT], FP8)

            wsrc = big.tile([P, 16], FP8)
            vpa = big.tile([P, HC], BF16)        # warm-keeper pacer chain
            vpb = big.tile([P, HC], BF16)

            # warm-up collective: identical shape to the real AllGathers,
            # fired during the stream phase so the first real AG doesn't
            # pay the ~15 us ncfw arming delay
            nc.gpsimd.memset(wsrc[:], 0)
            aginW = dram.tile([P, 16], FP8, tag="aginW")
            agoutW = dram.tile([NCORES, P, 16], FP8,
                               addr_space="Shared", tag="agoutW")
            nc.gpsimd.dma_start(aginW[:], wsrc[:])
            nc.gpsimd.collective_compute(
                "AllGather", ALU.bypass,
                replica_groups=[list(range(NCORES))],
                ins=[aginW[:]], outs=[agoutW[:]],
            )

            # pad rows of the raw-output tiles are read (as garbage) by
            # the full-128 transposes; define them once (base must be
            # 32-aligned, so start at 64 — rows 64..71 are re-written by
            # every psum evacuation afterwards)
            for t in (y1raw, x2raw, y1craw, x2craw):
                nc.vector.memset(t[64:P, :], 0)
            nc.vector.memset(vpa[:], 0)
            nc.vector.memset(vpb[:], 0)


            # ---------- input DMAs ----------
            nc.scalar.dma_start(z0[:], z0_d[:])
            nc.scalar.dma_start(x0T[:], x0T_d[:])
            nc.scalar.dma_start(hxT[:], hxT_d[:])
            for t, s in ((wr0, wr0_d), (wu0, wu0_d), (wc0, wc0_d),
                         (wr12, wr12_d), (wu12, wu12_d), (wc12, wc12_d),
                         (brur, brur_d), (bruu, bruu_d), (bc, bc_d),
                         (cdzA, cdzA_d), (cdzB, cdzB_d),
                         (identbf, ident_d)):
                nc.gpsimd.dma_start(t[:], s[:])

            # ---------- adjacency stream (fp8, p-major contiguous) ----
            S, G, Csc = nc.sync, nc.gpsimd, nc.scalar
            ENGS = [S, G, Csc, S, G, S, G, Csc, S, G, S, Csc, G, S, G, Csc]
            for i in range(NT):
                ENGS[i].dma_start(abf[i][:], a_d[:, i * CPI:(i + 1) * CPI, :])

            # DoubleRow chunk pairs: (4i+lo, 4i+2+lo) — both chunks in
            # stream tile i, Ko step 2*FT = 144 bytes (16-aligned), so the
            # z tiles stay tight 72-wide and the gather lands contiguously.
            def zpair(zt, i, lo):
                v = zt[:].rearrange("p (i hi lo) f -> p i hi lo f",
                                    hi=2, lo=2)
                return v[:, i, :, lo, :]

            def apair(i, lo, h):
                v = abf[i][:].rearrange("p (hi lo) c -> p hi lo c", hi=2)
                return v[:, :, lo, h * HC:(h + 1) * HC]

            def mm_half(ps, zt, h):
                for i in range(NT):
                    for lo in range(2):
                        nc.tensor.matmul(
                            ps[:], lhsT=zpair(zt, i, lo),
                            rhs=apair(i, lo, h),
                            start=(i == 0 and lo == 0),
                            stop=(i == NT - 1 and lo == 1),
                            perf_mode=DR,
                        )

            def transposes_packed(srcT, pk, cdz, mbs):
                """[P, C] node-block (rows 0..71 packed) -> [P, 72], *cdz."""
                for mb in mbs:
                    pt = pstp.tile([P, P], BF16, tag="tp")
                    nc.tensor.transpose(
                        pt[:], srcT[:, mb * P:(mb + 1) * P], identbf[:])
                    cdb = cdz[:, mb:mb + 1].broadcast_to((P, FT))
                    nc.vector.tensor_tensor(
                        pk[:, mb, :], pt[:, 0:FT], cdb, ALU.mult)

            def transposes_padded(srcT, pk, cdz, mbs):
                """Padded [P, C] node-block -> packed payload, *cdz."""
                for mb in mbs:
                    pt = pstp.tile([P, P], BF16, tag="tp")
                    nc.tensor.transpose(
                        pt[:], srcT[:, mb * P:(mb + 1) * P], identbf[:])
                    cdb = cdz[:, mb:mb + 1].unsqueeze(-1).broadcast_to(
                        (P, B, IT))
                    src = pt[:].rearrange("p (b e) -> p b e", b=B)[:, :, 0:IT]
                    nc.vector.tensor_tensor(
                        pk[:, mb, :, :], src, cdb, ALU.mult)

            def allgather(pk, zdst, name):
                agin = dram.tile([P, MB * FT], FP8, tag=f"agin{name}")
                agout = dram.tile([NCORES, P, MB * FT], FP8,
                                  addr_space="Shared", tag=f"agout{name}")
                half = MB // 2 * FT
                nc.gpsimd.dma_start(
                    agin[:, 0:half],
                    pk[:, 0:MB // 2].rearrange("p ... -> p (...)"))
                nc.gpsimd.dma_start(
                    agin[:, half:],
                    pk[:, MB // 2:].rearrange("p ... -> p (...)"))
                nc.gpsimd.collective_compute(
                    "AllGather", ALU.bypass,
                    replica_groups=[list(range(NCORES))],
                    ins=[agin[:]], outs=[agout[:]],
                )
                gengs = (nc.sync, nc.sync, nc.sync, nc.scalar, nc.scalar,
                         nc.scalar, nc.gpsimd, nc.gpsimd)
                for r in range(NCORES):
                    gengs[r].dma_start(
                        zdst[:, r * MB:(r + 1) * MB, :],
                        agout[r].rearrange("p (m f) -> p m f", m=MB),
                    )

            def gate_m0(w0, pg, h):
                fs = slice(h * HC, (h + 1) * HC)
                for b in range(B):
                    nc.tensor.matmul(
                        pg[b * 32:(b + 1) * 32, :],
                        lhsT=w0[b * 32:b * 32 + IT, :],
                        rhs=x0T[b * 32:b * 32 + IT, fs],
                        start=True, stop=False,
                        tile_position=(b * 32, b * 32),
                    )

            def gate_m12(w12, m, pg, src, h, stop):
                fs = slice(h * HC, (h + 1) * HC)
                nc.tensor.matmul(
                    pg[:], lhsT=w12[:, m, :], rhs=src[0:FT, fs],
                    start=False, stop=stop,
                )

            def keep_warm(rounds, pd):
                """Pace tiny PE matmul pulses across an AllGather window
                (~2.5 us apart) so the HAM clock-gate stays at 8/8.  The
                clock is a serial DVE copy chain on the otherwise-idle
                vector engine; each pulse matmul reads the chain tile so
                it cannot run before its tick."""
                pp = pd.partition_size()
                t = [vpa, vpb]
                for k in range(rounds):
                    src_t, dst_t = t[k % 2], t[(k + 1) % 2]
                    for _ in range(6):
                        nc.vector.tensor_copy(dst_t[:], src_t[:])
                        src_t, dst_t = dst_t, src_t
                    nc.tensor.matmul(
                        pd[:, 0:P], lhsT=identbf[:, 0:pp],
                        rhs=src_t[:, 0:P],
                        start=True, stop=True, skip_group_check=True,
                    )

            # ================= gconv 1 (r/u gates) =================
            # diff A chases the stream (h-inner: consume tiles as they land)
            psA = [psmm.tile([FT, HC], F32, tag="mm", name=f"psA{h}")
                   for h in range(2)]
            for i in range(NT):
                for lo in range(2):
                    for h in range(2):
                        nc.tensor.matmul(
                            psA[h][:], lhsT=zpair(z0, i, lo),
                            rhs=apair(i, lo, h),
                            start=(i == 0 and lo == 0),
                            stop=(i == NT - 1 and lo == 1),
                            perf_mode=DR,
                        )
            for h in range(2):
                nc.vector.tensor_copy(
                    y1raw[0:FT, h * HC:(h + 1) * HC], psA[h][:])
            transposes_packed(y1raw, pkA, cdzA, range(MB))
            allgather(pkA, zgA, "A")
            # r/u gate m0+m1 run inside the AG window
            pg_r = [psg.tile([P, HC], F32, tag="gate", name=f"pgr{h}",
                             bufs=4) for h in range(2)]
            pg_u = [psg.tile([P, HC], F32, tag="gate", name=f"pgu{h}",
                             bufs=4) for h in range(2)]
            for h in range(2):
                gate_m0(wr0, pg_r[h], h)
                gate_m12(wr12, 0, pg_r[h], y1raw, h, False)
                gate_m0(wu0, pg_u[h], h)
                gate_m12(wu12, 0, pg_u[h], y1raw, h, False)
            pdA = psmm.tile([FT, HC], F32, tag="mm", name="pdA")
            keep_warm(12, pdA)

            # diff B (h-outer: half-0 tail overlaps half-1 matmuls)
            psB = [psmm.tile([FT, HC], F32, tag="mm", name=f"psB{h}")
                   for h in range(2)]
            for h in range(2):
                fs = slice(h * HC, (h + 1) * HC)
                mm_half(psB[h], zgA, h)
                nc.vector.tensor_copy(x2raw[0:FT, fs], psB[h][:])
                gate_m12(wr12, 1, pg_r[h], x2raw, h, True)
                gate_m12(wu12, 1, pg_u[h], x2raw, h, True)
                nc.scalar.activation(sigR[:, fs], pg_r[h][:], AF.Sigmoid,
                                     bias=brur[:])
                for b in range(B):
                    nc.vector.tensor_tensor(
                        x0T[b * 32:b * 32 + U, fs],
                        sigR[b * 32:b * 32 + U, fs],
                        hxT[b * 32:b * 32 + U, fs],
                        ALU.mult,
                    )
                transposes_padded(x0T, pkB, cdzB, range(h * MB // 2,
                                                        (h + 1) * MB // 2))
                nc.scalar.activation(sigU[:, fs], pg_u[h][:], AF.Sigmoid,
                                     bias=bruu[:])
            allgather(pkB, zgB, "B")
            # c gate m0 fills part of the AG_B window (x0T is x0cT now)
            pg_c = [psg.tile([P, HC], F32, tag="gate", name=f"pgc{h}",
                             bufs=4) for h in range(2)]
            for h in range(2):
                gate_m0(wc0, pg_c[h], h)
            pdB = psg.tile([P, HC], F32, tag="gate", name="pdB", bufs=4)
            keep_warm(6, pdB)

            # ================= gconv 2 (candidate c) =================
            psC = [psmm.tile([FT, HC], F32, tag="mm", name=f"psC{h}")
                   for h in range(2)]
            for h in range(2):
                fs = slice(h * HC, (h + 1) * HC)
                mm_half(psC[h], zgB, h)
                nc.vector.tensor_copy(y1craw[0:FT, fs], psC[h][:])
                transposes_packed(y1craw, pkC, cdzA,
                                  range(h * MB // 2, (h + 1) * MB // 2))
            allgather(pkC, zgA, "C")
            for h in range(2):
                gate_m12(wc12, 0, pg_c[h], y1craw, h, False)
            pdC = psg.tile([P, HC], F32, tag="gate", name="pdC", bufs=4)
            keep_warm(7, pdC)

            # diff D + per-half tail to the output DMA
            psD = [psmm.tile([FT, HC], F32, tag="mm", name=f"psD{h}")
                   for h in range(2)]
            for h in range(2):
                fs = slice(h * HC, (h + 1) * HC)
                mm_half(psD[h], zgA, h)
                nc.vector.tensor_copy(x2craw[0:FT, fs], psD[h][:])
                gate_m12(wc12, 1, pg_c[h], x2craw, h, True)
                # out = c + u*(h - c), pipelined per 256-col quarter
                for q in range(2):
                    qs = slice(h * HC + q * 256, h * HC + (q + 1) * 256)
                    qp = slice(q * 256, (q + 1) * 256)
                    nc.scalar.activation(cT[:, qs], pg_c[h][:, qp],
                                         AF.Tanh, bias=bc[:])
                    eng = nc.gpsimd if h == 0 else nc.vector
                    eng.tensor_tensor(outT[:, qs], hxT[:, qs], cT[:, qs],
                                      ALU.subtract)
                    eng.tensor_tensor(outT[:, qs], outT[:, qs],
                                      sigU[:, qs], ALU.mult)
                    eng.tensor_tensor(outT[:, qs], outT[:, qs], cT[:, qs],
                                      ALU.add)
                    (nc.sync if h == 0 else nc.scalar).dma_start(
                        out_d[:, qs], outT[:, qs])

    nc.compile()
    return nc


def _get_nc():
    if "nc" not in _CACHE:
        _CACHE["nc"] = _build()
    return _CACHE["nc"]


# feature permutation: device feature j -> reference feature i
# j = 0..15 -> i = j+2 (hidden), j = 16,17 -> i = j-16 (input x)
_PERM = np.array(list(range(2, 18)) + [0, 1])


def _host_prep(inputs, hx, adj, W_ru, b_ru, W_c, b_c):
    f32 = np.float32
    bf16 = ml_dtypes.bfloat16
    fp8 = ml_dtypes.float8_e4m3fn

    xr = np.ascontiguousarray(inputs, dtype=f32).reshape(B, N, IN_DIM)
    hr = np.ascontiguousarray(hx, dtype=f32).reshape(B, N, U)
    x0 = np.concatenate([hr, xr], axis=2).transpose(1, 0, 2)  # [N, B, 18]
    x0 = np.ascontiguousarray(x0).reshape(N, FT)

    adj = np.asarray(adj, f32)
    d = adj.sum(axis=1) + 1.0
    dinv = 1.0 / d

    # z0 = C0 * dinv * x0, tight 72-wide, p-major, fp8
    z0 = (C0 * dinv)[:, None] * x0                  # [N, 72]
    z0 = z0.reshape(KCH, P, FT).transpose(1, 0, 2)  # [P, KCH, FT]
    z0 = np.ascontiguousarray(z0).astype(fp8)

    def w_split(w, lo):
        # fold the diffusion unscales into the weights:
        # gate = x0*(W0-W2) + (c0*x1)*(W1/c0) + (c1/2*Tx1)*(2*W2/c1)
        w3 = np.asarray(w, f32).reshape(IT, 3, -1)[_PERM][:, :, lo:lo + U]
        w0 = w3[:, 0] - w3[:, 2]                    # [18, U]
        m0 = np.zeros((B, 32, 32), f32)
        m0[:, 0:IT, 0:U] = w0[None]
        w1 = S0 * w3[:, 1]
        w2 = S1 * w3[:, 2]
        m12 = np.zeros((B, IT, 2, B, 32), f32)
        for b in range(B):
            m12[b, :, 0, b, 0:U] = w1
            m12[b, :, 1, b, 0:U] = w2
        return (m0.reshape(P, 32).astype(bf16),
                m12.reshape(FT, 2, P).astype(bf16))

    wr0_p, wr12_p = w_split(W_ru, 0)
    wu0_p, wu12_p = w_split(W_ru, U)
    wc0_p, wc12_p = w_split(W_c, 0)

    def pad_bias(v):
        t = np.zeros((B, 32), f32)
        t[:, 0:U] = np.asarray(v, f32)
        return np.ascontiguousarray(t.reshape(P)[:, None])

    brur_t = pad_bias(np.asarray(b_ru, f32)[0:U])
    bruu_t = pad_bias(np.asarray(b_ru, f32)[U:2 * U])
    bc_t = pad_bias(np.asarray(b_c, f32))
    ident = np.eye(P, dtype=f32).astype(bf16)

    in_maps = []
    for m in range(NCORES):
        sl = slice(m * C, (m + 1) * C)
        a_m = np.ascontiguousarray(adj[:, sl])
        a_m[m * C + np.arange(C), np.arange(C)] += 1.0
        # p-major: [p, k, c] = row k*128+p
        a_m = a_m.reshape(KCH, P, C).transpose(1, 0, 2)
        a_m = np.ascontiguousarray(a_m).astype(fp8)

        x0own = x0[sl]
        x0T = np.zeros((B, 32, C), f32)
        x0T[:, 0:IT, :] = x0own.reshape(C, B, IT).transpose(1, 2, 0)
        x0T = x0T.reshape(P, C).astype(bf16)
        hxT_p = np.zeros((B, 32, C), f32)
        hxT_p[:, 0:U, :] = hr[:, sl, :].transpose(0, 2, 1)
        hxT_p = hxT_p.reshape(P, C).astype(bf16)
        # per-own-node payload scales, node-major [p, mb]
        dlocal = dinv[sl].reshape(MB, P).T
        cdzA_p = np.ascontiguousarray((C1 / C0) * dlocal).astype(bf16)
        cdzB_p = np.ascontiguousarray(C0 * dlocal).astype(bf16)
        in_maps.append({
            "a": a_m,
            "z0": z0,
            "x0T": x0T,
            "hxT": hxT_p,
            "wr0": wr0_p,
            "wu0": wu0_p,
            "wc0": wc0_p,
            "wr12": wr12_p,
            "wu12": wu12_p,
            "wc12": wc12_p,
            "brur": brur_t,
            "bruu": bruu_t,
            "bc": bc_t,
            "cdzA": cdzA_p,
            "cdzB": cdzB_p,
            "ident": ident,
        })
    return in_maps


def _run(in_maps, trace=False, **kw):
    nc = _get_nc()
    return bass_utils.run_bass_kernel_spmd(
        nc, in_maps, core_ids=list(range(NCORES)), trace=trace, **kw)


def _assemble(results):
    out = np.empty((B, N * U), np.float32)
    for m in range(NCORES):
        # device layout [b*32+u, n] (rows 16..31 per block are padding)
        blk = results[m]["out"].astype(np.float32).reshape(B, 32, C)[:, 0:U, :].transpose(0, 2, 1)
        out[:, m * C * U:(m + 1) * C * U] = blk.reshape(B, C * U)
    return out


def kernel(inputs, hx, adj, W_ru, b_ru, W_c, b_c):
    in_maps = _host_prep(inputs, hx, adj, W_ru, b_ru, W_c, b_c)
    res = _run(in_maps)
    return _assemble(res.results)
